# revision 62
# baseline (speedup 1.0000x reference)
"""Trainium2 Bass kernel: aspect-level sentiment classification head.

  aspect[b] = mean(last_hidden_state[b, start_b:end_b, :])   (ragged spans)
  out = concat([pooled, aspect], -1) @ W.T + b

Strategy: data-parallel over batch (8 samples per core, 8 cores).  The key
observation is that only the span rows of last_hidden_state are ever needed,
so each core *gathers* just those rows from DRAM with an indirect DMA whose
row indices are computed on-device from position_indices.  Spans are padded
to L = 32*m rows (m = power of two chosen from the max span length at call
time); rows past the span end are masked to zero.  The per-sample 1/len is
folded into the mask so a single PE matmul per 128-column chunk produces the
*transposed* aspect features directly, which then feed an accumulated
12-chunk GEMM against host-pre-transposed W.
"""

import os
import sys

if "/opt/trn_rl_repo" not in sys.path:
    sys.path.insert(0, "/opt/trn_rl_repo")

import numpy as np

import concourse.bass as bass
import concourse.tile as tile
from concourse import bacc, mybir
from concourse.bass import IndirectOffsetOnAxis
from concourse.bass_utils import run_bass_kernel_spmd

F32 = mybir.dt.float32
I32 = mybir.dt.int32
BF16 = mybir.dt.bfloat16

B, S, H, C = 64, 4096, 768, 3
NCORES = 8
BL = B // NCORES          # samples per core
P = 128
HC = H // P               # 6 hidden chunks of 128
KC = 2 * H // P           # 12 contraction chunks in the final GEMM


def _log2(x: int) -> int:
    l = x.bit_length() - 1
    assert 1 << l == x
    return l


def build(m: int):
    """Build + compile the per-core SPMD program for spans up to 32*m rows."""
    assert m & (m - 1) == 0 and 1 <= m <= S // 32
    nblk = BL * m            # 32-row blocks per core
    G = nblk // 4            # gather groups of 128 rows
    cols = max(1, 4 // m)    # samples covered by one group
    gps = max(1, m // 4)     # groups per sample
    lm = _log2(m)

    # packed host-side constants: per-group sample-indicator matrices for the
    # PE broadcast (rows 0-7), jrow / sample-base ramps, block masks, and the
    # identity for the param transposes -- all pure functions of (m, p, g)
    c_jrow = 128 * G
    c_s4 = c_jrow + G
    c_blk = c_s4 + G
    c_id = c_blk + cols * G
    CW = c_id + 48

    nc = bacc.Bacc("TRN2", target_bir_lowering=False, debug=False,
                   num_devices=NCORES)
    lhs = nc.dram_tensor("lhs", [BL * S, H], F32, kind="ExternalInput").ap()
    pooled_r = nc.dram_tensor("pooled_r", [HC * BL, P], F32,
                              kind="ExternalInput").ap()
    w_r = nc.dram_tensor("w_r", [KC * C, P], F32, kind="ExternalInput").ap()
    pos = nc.dram_tensor("pos", [BL, 2], I32, kind="ExternalInput").ap()
    bias = nc.dram_tensor("bias", [BL, C], F32, kind="ExternalInput").ap()
    consts = nc.dram_tensor("consts", [P, CW], F32, kind="ExternalInput").ap()
    out = nc.dram_tensor("out", [BL, C], F32, kind="ExternalOutput").ap()

    with tile.TileContext(nc) as tc:
        packed = m <= 4  # one PSUM bank for all 6 aspect chunks vs 6 banks
        with (
            tc.tile_pool(name="const", bufs=1) as cp,
            tc.tile_pool(name="work", bufs=4) as wp,
            tc.tile_pool(name="rows", bufs=4) as rp,
            tc.tile_pool(name="pmisc", bufs=1, space="PSUM") as pm,
            tc.tile_pool(name="pbc", bufs=2 if packed else 1,
                         space="PSUM") as pb,
            tc.tile_pool(name="pasp", bufs=1, space="PSUM") as pa,
        ):
            # ---- constants / params -------------------------------------
            consts_sb = cp.tile([P, CW], F32, tag="consts_sb")
            nc.scalar.dma_start(consts_sb[:], consts[:, :])
            id48 = consts_sb[0:48, c_id:c_id + 48]

            pos_i = cp.tile([BL, 2], I32, tag="pos_i")
            nc.sync.dma_start(pos_i[:], pos[:, :])
            pos_f = cp.tile([BL, 2], F32, tag="pos_f")
            nc.vector.tensor_copy(pos_f[:], pos_i[:])

            pooled_sb = cp.tile([HC * BL, P], F32, tag="pooled_sb")
            nc.sync.dma_start(pooled_sb[:], pooled_r[:, :])
            w_sb = cp.tile([KC * C, P], F32, tag="w_sb")
            nc.sync.dma_start(w_sb[:], w_r[:, :])
            bias_sb = cp.tile([BL, C], F32, tag="bias_sb")
            nc.sync.dma_start(bias_sb[:], bias[:, :])

            # transpose pooled_r -> pT [128, 48] (pT[h, c*8+b] = pooled[b, c*128+h])
            pT_ps = pm.tile([P, HC * BL], F32, tag="pmisc", name="pT_ps")
            nc.tensor.transpose(pT_ps[:], pooled_sb[:], id48)
            pT = cp.tile([P, HC * BL], F32, tag="pT")
            nc.vector.tensor_copy(pT[:], pT_ps[:])

            # transpose w_r -> wT [128, 36] (wT[h, c*3+j] = W[j, c*128+h])
            wT_ps = pm.tile([P, KC * C], F32, tag="pmisc", name="wT_ps")
            nc.tensor.transpose(wT_ps[:], w_sb[:],
                                consts_sb[0:KC * C, c_id:c_id + KC * C])
            wT = cp.tile([P, KC * C], F32, tag="wT")
            nc.vector.tensor_copy(wT[:], wT_ps[:])

            # psum accumulators for transposed aspect features; for m >= 8
            # accumulation groups stay open across gather groups, so each
            # hidden chunk needs its own bank
            if packed:
                aspT_all = pa.tile([P, HC * BL], F32, tag="aspT")
                aspT_ps = [aspT_all[:, c * BL:(c + 1) * BL]
                           for c in range(HC)]
            else:
                aspT_ps = [pa.tile([P, BL], F32, tag=f"aspT{c}",
                                   name=f"aspT{c}")[:] for c in range(HC)]

            # ---- gather groups ------------------------------------------
            for g in range(G):
                # broadcast (start, end) of each partition's sample via PE
                # using the host-provided indicator ind[s, p] = 1 iff
                # s == (4g + p//32) >> lm
                ind = consts_sb[0:BL, g * 128:(g + 1) * 128]
                bc_ps = pb.tile([P, 2], F32, tag="bc")
                nc.tensor.matmul(out=bc_ps[:], lhsT=ind, rhs=pos_f[:],
                                 start=True, stop=True)
                bc = wp.tile([P, 2], F32, tag="bcs")
                nc.vector.tensor_copy(bc[:], bc_ps[:])
                st_f = bc[:, 0:1]
                en_f = bc[:, 1:2]

                # host-provided per-group ramps
                jrow_f = consts_sb[:, c_jrow + g:c_jrow + g + 1]
                s4096_f = consts_sb[:, c_s4 + g:c_s4 + g + 1]

                # gather row index = min(start + jrow, S-1) + 4096*s
                row_f = wp.tile([P, 1], F32, tag="row_f")
                nc.vector.tensor_add(row_f[:], st_f, jrow_f)
                idx_f = wp.tile([P, 1], F32, tag="idx_f")
                nc.vector.tensor_scalar(
                    idx_f[:], row_f[:], float(S - 1), s4096_f,
                    mybir.AluOpType.min, mybir.AluOpType.add)
                idx_i = wp.tile([P, 1], I32, tag="idx_i")
                nc.vector.tensor_copy(idx_i[:], idx_f[:])

                # mask = (jrow < len) / len  (len==0 -> NaN, matches 0/0 ref)
                len_f = wp.tile([P, 1], F32, tag="len_f")
                nc.vector.tensor_sub(len_f[:], en_f, st_f)
                recip = wp.tile([P, 1], F32, tag="recip")
                nc.vector.reciprocal(recip[:], len_f[:])
                inm = wp.tile([P, 1], F32, tag="inm")
                nc.vector.tensor_tensor(out=inm[:], in0=jrow_f, in1=len_f[:],
                                        op=mybir.AluOpType.is_lt)
                inm_s = wp.tile([P, 1], F32, tag="inm_s")
                nc.vector.tensor_mul(inm_s[:], inm[:], recip[:])

                # bf16 mask: per-sample column pattern applied in one op
                mk = wp.tile([P, cols], BF16, tag="mk")
                nc.vector.tensor_tensor(
                    out=mk[:], in0=inm_s[:, 0:1].to_broadcast([P, cols]),
                    in1=consts_sb[:, c_blk + g * cols:c_blk + (g + 1) * cols],
                    op=mybir.AluOpType.mult)
                maskg = mk[:]

                rows_t = rp.tile([P, H], BF16, tag="rows")
                nc.gpsimd.indirect_dma_start(
                    out=rows_t[:], out_offset=None, in_=lhs[:, :],
                    in_offset=IndirectOffsetOnAxis(ap=idx_i[:, 0:1], axis=0))

                # aspT[h, s] += rows[:, chunk].T @ mask
                s_lo = (4 * g) // m
                first = g % gps == 0
                last = g % gps == gps - 1
                for c in range(HC):
                    nc.tensor.matmul(
                        out=aspT_ps[c][:, s_lo:s_lo + cols],
                        lhsT=rows_t[:, c * P:(c + 1) * P], rhs=maskg,
                        start=first, stop=last)

            # ---- final GEMM: out[b, j] = sum_k featT[k, b] * wT[k, j] ----
            aspT_sb = cp.tile([P, HC * BL], F32, tag="aspT_sb")
            if packed:
                nc.vector.tensor_copy(aspT_sb[:], aspT_all[:])
            else:
                for c in range(HC):
                    nc.vector.tensor_copy(aspT_sb[:, c * BL:(c + 1) * BL],
                                          aspT_ps[c])

            out_ps = pm.tile([BL, C], F32, tag="pmisc", name="out_ps")
            for c in range(KC):
                featT = (pT[:, (c * BL):(c + 1) * BL] if c < HC
                         else aspT_sb[:, (c - HC) * BL:(c - HC + 1) * BL])
                nc.tensor.matmul(out=out_ps[:], lhsT=featT,
                                 rhs=wT[:, c * C:(c + 1) * C],
                                 start=(c == 0), stop=(c == KC - 1))

            out_sb = cp.tile([BL, C], F32, tag="out_sb")
            nc.vector.tensor_add(out_sb[:], out_ps[:], bias_sb[:])
            nc.sync.dma_start(out[:, :], out_sb[:])

    nc.compile()
    return nc


_CACHE: dict[int, object] = {}


def _get(m: int):
    if m not in _CACHE:
        _CACHE[m] = build(m)
    return _CACHE[m]


def kernel(last_hidden_state, pooled_output, position_indices, W, b):
    last_hidden_state = np.ascontiguousarray(last_hidden_state,
                                             dtype=np.float32)
    pooled_output = np.ascontiguousarray(pooled_output, dtype=np.float32)
    position_indices = np.ascontiguousarray(position_indices, dtype=np.int32)
    W = np.ascontiguousarray(W, dtype=np.float32)
    b = np.ascontiguousarray(b, dtype=np.float32)

    lens = position_indices[:, 1] - position_indices[:, 0]
    maxlen = max(1, int(lens.max()))
    m = 1
    while 32 * m < maxlen:
        m *= 2
    nc = _get(m)

    in_maps = _make_in_maps(m, last_hidden_state, pooled_output,
                            position_indices, W, b)
    res = run_bass_kernel_spmd(nc, in_maps, core_ids=list(range(NCORES)),
                               **RUN_KWARGS)
    global LAST_RESULT
    LAST_RESULT = res
    return np.concatenate([res.results[c]["out"] for c in range(NCORES)],
                          axis=0)


def _make_in_maps(m, last_hidden_state, pooled_output, position_indices,
                  W, b):
    G = BL * m // 4
    cols = max(1, 4 // m)
    lm = _log2(m)
    c_jrow = 128 * G
    c_s4 = c_jrow + G
    c_blk = c_s4 + G
    c_id = c_blk + cols * G
    CW = c_id + 48

    p = np.arange(P)
    consts = np.zeros((P, CW), np.float32)
    for g in range(G):
        k = 4 * g + p // 32
        s_of_p = k >> lm
        consts[s_of_p, g * 128 + p] = 1.0
        consts[p, c_jrow + g] = (k & (m - 1)) * 32 + p % 32
        consts[p, c_s4 + g] = s_of_p * S
        s_lo = (4 * g) // m
        for j in range(cols):
            consts[:, c_blk + g * cols + j] = (s_of_p == s_lo + j)
    consts[0:48, c_id:c_id + 48] = np.eye(48, dtype=np.float32)

    w_r = np.ascontiguousarray(
        W.reshape(C, KC, P).transpose(1, 0, 2).reshape(KC * C, P))
    bias_t = np.ascontiguousarray(np.broadcast_to(b[None, :], (BL, C)))
    in_maps = []
    for cid in range(NCORES):
        sl = slice(cid * BL, (cid + 1) * BL)
        in_maps.append({
            "lhs": last_hidden_state[sl].reshape(BL * S, H),
            "pooled_r": np.ascontiguousarray(
                pooled_output[sl].reshape(BL, HC, P).transpose(1, 0, 2)
                .reshape(HC * BL, P)),
            "pos": position_indices[sl],
            "w_r": w_r,
            "bias": bias_t,
            "consts": consts,
        })
    return in_maps


# test/bench hooks (harness just calls kernel(); these stay default)
RUN_KWARGS: dict = {}
LAST_RESULT = None


# revision 67
# speedup vs baseline: 1.0219x; 1.0219x over previous
"""Trainium2 Bass kernel: aspect-level sentiment classification head.

  aspect[b] = mean(last_hidden_state[b, start_b:end_b, :])   (ragged spans)
  out = concat([pooled, aspect], -1) @ W.T + b

Strategy: data-parallel over batch (8 samples per core, 8 cores).  The key
observation is that only the span rows of last_hidden_state are ever needed,
so each core *gathers* just those rows from DRAM with an indirect DMA whose
row indices are computed on-device from position_indices.  Spans are padded
to L = 32*m rows (m = power of two chosen from the max span length at call
time); rows past the span end are masked to zero.  The per-sample 1/len is
folded into the mask so a single PE matmul per 128-column chunk produces the
*transposed* aspect features directly, which then feed an accumulated
12-chunk GEMM against host-pre-transposed W.
"""

import os
import sys

if "/opt/trn_rl_repo" not in sys.path:
    sys.path.insert(0, "/opt/trn_rl_repo")

import numpy as np

import concourse.bass as bass
import concourse.tile as tile
from concourse import bacc, mybir
from concourse.bass import IndirectOffsetOnAxis
from concourse.bass_utils import run_bass_kernel_spmd

F32 = mybir.dt.float32
I32 = mybir.dt.int32
BF16 = mybir.dt.bfloat16

B, S, H, C = 64, 4096, 768, 3
NCORES = 8
BL = B // NCORES          # samples per core
P = 128
HC = H // P               # 6 hidden chunks of 128
KC = 2 * H // P           # 12 contraction chunks in the final GEMM


def _log2(x: int) -> int:
    l = x.bit_length() - 1
    assert 1 << l == x
    return l


def build(m: int):
    """Build + compile the per-core SPMD program for spans up to 32*m rows.

    Each gather descriptor fetches a *pair* of consecutive rows (1536
    contiguous elements), so one 128-partition indirect DMA covers 256 rows:
    for the common m=1 case the whole core needs a single gather."""
    assert m & (m - 1) == 0 and 1 <= m <= S // 32
    G = m                    # gather groups of 128 row-pairs
    cols = max(1, 8 // m)    # samples covered by one group
    gps = max(1, m // 8)     # groups per sample
    lm = _log2(m)

    # packed host-side constants: per-group sample-indicator matrices for the
    # PE broadcast (rows 0-7), jrow / sample-base ramps, block masks, and the
    # identity for the param transposes -- all pure functions of (m, p, g)
    c_jrow = 128 * G
    c_s4 = c_jrow + G
    c_blk = c_s4 + G
    c_id = c_blk + cols * G
    CW = c_id + 48

    nc = bacc.Bacc("TRN2", target_bir_lowering=False, debug=False,
                   num_devices=NCORES)
    lhs = nc.dram_tensor("lhs", [BL * S, H], F32, kind="ExternalInput").ap()
    pooled_r = nc.dram_tensor("pooled_r", [HC * BL, P], F32,
                              kind="ExternalInput").ap()
    w_r = nc.dram_tensor("w_r", [KC * C, P], F32, kind="ExternalInput").ap()
    pos = nc.dram_tensor("pos", [BL, 2], I32, kind="ExternalInput").ap()
    bias = nc.dram_tensor("bias", [BL, C], F32, kind="ExternalInput").ap()
    consts = nc.dram_tensor("consts", [P, CW], F32, kind="ExternalInput").ap()
    out = nc.dram_tensor("out", [BL, C], F32, kind="ExternalOutput").ap()

    with tile.TileContext(nc) as tc:
        packed = m <= 8  # one PSUM bank for all 6 aspect chunks vs 6 banks
        with (
            tc.tile_pool(name="const", bufs=1) as cp,
            tc.tile_pool(name="work", bufs=4) as wp,
            tc.tile_pool(name="rows", bufs=4) as rp,
            tc.tile_pool(name="pmisc", bufs=1, space="PSUM") as pm,
            tc.tile_pool(name="pbc", bufs=2 if packed else 1,
                         space="PSUM") as pb,
            tc.tile_pool(name="pasp", bufs=1, space="PSUM") as pa,
        ):
            # ---- constants / params -------------------------------------
            consts_sb = cp.tile([P, CW], F32, tag="consts_sb")
            nc.scalar.dma_start(consts_sb[:], consts[:, :])
            id48 = consts_sb[0:48, c_id:c_id + 48]

            pos_i = cp.tile([BL, 2], I32, tag="pos_i")
            nc.sync.dma_start(pos_i[:], pos[:, :])
            pos_f = cp.tile([BL, 2], F32, tag="pos_f")
            nc.vector.tensor_copy(pos_f[:], pos_i[:])

            pooled_sb = cp.tile([HC * BL, P], F32, tag="pooled_sb")
            nc.sync.dma_start(pooled_sb[:], pooled_r[:, :])
            w_sb = cp.tile([KC * C, P], F32, tag="w_sb")
            nc.sync.dma_start(w_sb[:], w_r[:, :])
            bias_sb = cp.tile([BL, C], F32, tag="bias_sb")
            nc.sync.dma_start(bias_sb[:], bias[:, :])

            # transpose pooled_r -> pT [128, 48] (pT[h, c*8+b] = pooled[b, c*128+h])
            pT_ps = pm.tile([P, HC * BL], F32, tag="pmisc", name="pT_ps")
            nc.tensor.transpose(pT_ps[:], pooled_sb[:], id48)
            pT = cp.tile([P, HC * BL], F32, tag="pT")
            nc.vector.tensor_copy(pT[:], pT_ps[:])

            # transpose w_r -> wT [128, 36] (wT[h, c*3+j] = W[j, c*128+h])
            wT_ps = pm.tile([P, KC * C], F32, tag="pmisc", name="wT_ps")
            nc.tensor.transpose(wT_ps[:], w_sb[:],
                                consts_sb[0:KC * C, c_id:c_id + KC * C])
            wT = cp.tile([P, KC * C], F32, tag="wT")
            nc.vector.tensor_copy(wT[:], wT_ps[:])

            # psum accumulators for transposed aspect features; for m >= 8
            # accumulation groups stay open across gather groups, so each
            # hidden chunk needs its own bank
            if packed:
                aspT_all = pa.tile([P, HC * BL], F32, tag="aspT")
                aspT_ps = [aspT_all[:, c * BL:(c + 1) * BL]
                           for c in range(HC)]
            else:
                aspT_ps = [pa.tile([P, BL], F32, tag=f"aspT{c}",
                                   name=f"aspT{c}")[:] for c in range(HC)]

            # ---- gather groups (128 row-pairs each) ----------------------
            for g in range(G):
                # broadcast (start, end) of each partition's sample via PE
                # using the host indicator ind[s, p] = 1 iff
                # s == (128g + p) >> (4 + lm)
                ind = consts_sb[0:BL, g * 128:(g + 1) * 128]
                bc_ps = pb.tile([P, 2], F32, tag="bc")
                nc.tensor.matmul(out=bc_ps[:], lhsT=ind, rhs=pos_f[:],
                                 start=True, stop=True)
                bc = wp.tile([P, 2], F32, tag="bcs")
                nc.vector.tensor_copy(bc[:], bc_ps[:])
                st_f = bc[:, 0:1]
                en_f = bc[:, 1:2]

                # host ramps: even-row offset 2t and sample base 4096*s
                jrow_f = consts_sb[:, c_jrow + g:c_jrow + g + 1]
                s4096_f = consts_sb[:, c_s4 + g:c_s4 + g + 1]

                # pair base row rc = min(start + 2t, S-2); idx = rc + 4096*s
                row_f = wp.tile([P, 1], F32, tag="row_f")
                nc.vector.tensor_add(row_f[:], st_f, jrow_f)
                rc_f = wp.tile([P, 1], F32, tag="rc_f")
                nc.vector.tensor_scalar(rc_f[:], row_f[:], float(S - 2),
                                        None, mybir.AluOpType.min)
                idx_f = wp.tile([P, 1], F32, tag="idx_f")
                nc.vector.tensor_tensor(out=idx_f[:], in0=rc_f[:],
                                        in1=s4096_f,
                                        op=mybir.AluOpType.add)
                idx_i = wp.tile([P, 1], I32, tag="idx_i")
                nc.vector.tensor_copy(idx_i[:], idx_f[:])

                # per-half masks: (start <= rc+h < end) / len  -- computed
                # from the clamped pair base so edge clamping stays exact
                len_f = wp.tile([P, 1], F32, tag="len_f")
                nc.vector.tensor_sub(len_f[:], en_f, st_f)
                recip = wp.tile([P, 1], F32, tag="recip")
                nc.vector.reciprocal(recip[:], len_f[:])
                mks = []
                for h in range(2):
                    rch = rc_f[:] if h == 0 else None
                    if h == 1:
                        rc1 = wp.tile([P, 1], F32, tag="rc1")
                        nc.vector.tensor_scalar(rc1[:], rc_f[:], 1.0, None,
                                                mybir.AluOpType.add)
                        rch = rc1[:]
                    ge = wp.tile([P, 1], F32, tag=f"ge{h}")
                    nc.vector.tensor_tensor(out=ge[:], in0=rch, in1=st_f,
                                            op=mybir.AluOpType.is_ge)
                    lt = wp.tile([P, 1], F32, tag=f"lt{h}")
                    nc.vector.tensor_tensor(out=lt[:], in0=rch, in1=en_f,
                                            op=mybir.AluOpType.is_lt)
                    inm = wp.tile([P, 1], F32, tag=f"inm{h}")
                    nc.vector.tensor_mul(inm[:], ge[:], lt[:])
                    inm_s = wp.tile([P, 1], F32, tag=f"inms{h}")
                    nc.vector.tensor_mul(inm_s[:], inm[:], recip[:])
                    mk = wp.tile([P, cols], BF16, tag=f"mk{h}")
                    nc.vector.tensor_tensor(
                        out=mk[:], in0=inm_s[:, 0:1].to_broadcast([P, cols]),
                        in1=consts_sb[:, c_blk + g * cols:
                                      c_blk + (g + 1) * cols],
                        op=mybir.AluOpType.mult)
                    mks.append(mk)

                rows_t = rp.tile([P, 2 * H], BF16, tag="rows")
                nc.gpsimd.indirect_dma_start(
                    out=rows_t[:], out_offset=None, in_=lhs[:, :],
                    in_offset=IndirectOffsetOnAxis(ap=idx_i[:, 0:1], axis=0))

                # aspT[h, s] += rows_half[:, chunk].T @ mask_half
                s_lo = (8 * g) // m
                first = g % gps == 0
                last = g % gps == gps - 1
                for c in range(HC):
                    for h in range(2):
                        nc.tensor.matmul(
                            out=aspT_ps[c][:, s_lo:s_lo + cols],
                            lhsT=rows_t[:, h * H + c * P:h * H + (c + 1) * P],
                            rhs=mks[h][:],
                            start=first and h == 0, stop=last and h == 1)

            # ---- final GEMM: out[b, j] = sum_k featT[k, b] * wT[k, j] ----
            aspT_sb = cp.tile([P, HC * BL], F32, tag="aspT_sb")
            if packed:
                nc.vector.tensor_copy(aspT_sb[:], aspT_all[:])
            else:
                for c in range(HC):
                    nc.vector.tensor_copy(aspT_sb[:, c * BL:(c + 1) * BL],
                                          aspT_ps[c])

            out_ps = pm.tile([BL, C], F32, tag="pmisc", name="out_ps")
            for c in range(KC):
                featT = (pT[:, (c * BL):(c + 1) * BL] if c < HC
                         else aspT_sb[:, (c - HC) * BL:(c - HC + 1) * BL])
                nc.tensor.matmul(out=out_ps[:], lhsT=featT,
                                 rhs=wT[:, c * C:(c + 1) * C],
                                 start=(c == 0), stop=(c == KC - 1))

            out_sb = cp.tile([BL, C], F32, tag="out_sb")
            nc.vector.tensor_add(out_sb[:], out_ps[:], bias_sb[:])
            nc.sync.dma_start(out[:, :], out_sb[:])

    nc.compile()
    return nc


_CACHE: dict[int, object] = {}


def _get(m: int):
    if m not in _CACHE:
        _CACHE[m] = build(m)
    return _CACHE[m]


def kernel(last_hidden_state, pooled_output, position_indices, W, b):
    last_hidden_state = np.ascontiguousarray(last_hidden_state,
                                             dtype=np.float32)
    pooled_output = np.ascontiguousarray(pooled_output, dtype=np.float32)
    position_indices = np.ascontiguousarray(position_indices, dtype=np.int32)
    W = np.ascontiguousarray(W, dtype=np.float32)
    b = np.ascontiguousarray(b, dtype=np.float32)

    lens = position_indices[:, 1] - position_indices[:, 0]
    maxlen = max(1, int(lens.max()))
    m = 1
    while 32 * m < maxlen:
        m *= 2
    nc = _get(m)

    in_maps = _make_in_maps(m, last_hidden_state, pooled_output,
                            position_indices, W, b)
    res = run_bass_kernel_spmd(nc, in_maps, core_ids=list(range(NCORES)),
                               **RUN_KWARGS)
    global LAST_RESULT
    LAST_RESULT = res
    return np.concatenate([res.results[c]["out"] for c in range(NCORES)],
                          axis=0)


def _make_in_maps(m, last_hidden_state, pooled_output, position_indices,
                  W, b):
    G = m
    cols = max(1, 8 // m)
    lm = _log2(m)
    c_jrow = 128 * G
    c_s4 = c_jrow + G
    c_blk = c_s4 + G
    c_id = c_blk + cols * G
    CW = c_id + 48

    p = np.arange(P)
    consts = np.zeros((P, CW), np.float32)
    for g in range(G):
        q = 128 * g + p                  # global row-pair index
        s_of_p = q >> (4 + lm)           # sample (16m pairs per sample)
        t = q & (16 * m - 1)             # pair within sample
        consts[s_of_p, g * 128 + p] = 1.0
        consts[p, c_jrow + g] = 2 * t
        consts[p, c_s4 + g] = s_of_p * S
        s_lo = (8 * g) // m
        for j in range(cols):
            consts[:, c_blk + g * cols + j] = (s_of_p == s_lo + j)
    consts[0:48, c_id:c_id + 48] = np.eye(48, dtype=np.float32)

    w_r = np.ascontiguousarray(
        W.reshape(C, KC, P).transpose(1, 0, 2).reshape(KC * C, P))
    bias_t = np.ascontiguousarray(np.broadcast_to(b[None, :], (BL, C)))
    in_maps = []
    for cid in range(NCORES):
        sl = slice(cid * BL, (cid + 1) * BL)
        in_maps.append({
            "lhs": last_hidden_state[sl].reshape(BL * S, H),
            "pooled_r": np.ascontiguousarray(
                pooled_output[sl].reshape(BL, HC, P).transpose(1, 0, 2)
                .reshape(HC * BL, P)),
            "pos": position_indices[sl],
            "w_r": w_r,
            "bias": bias_t,
            "consts": consts,
        })
    return in_maps


# test/bench hooks (harness just calls kernel(); these stay default)
RUN_KWARGS: dict = {}
LAST_RESULT = None
